# revision 27
# baseline (speedup 1.0000x reference)
"""Trainium2 Bass kernel for nn_MiddleLayerDecoderSplit.

Reference computation (per parent cluster p of N=4096, K=32 points):
    rel = x[:, :128] @ W_dec + b_dec            # [N, 96] -> [N*K, 3]
    h0  = concat(feat[cluster], rel_pts)        # [N*K, 259]
    h   = relu(relu(relu(h0@W1+b1)@W2+b2)@W3+b3)  # [N*K, 256]
Outputs: (rel_pts [N*K,3], h [N*K,256], cluster [N*K] int32)

Sharding: data-parallel over parent rows, 512 parents per core x 8 cores.
Weights replicated. cluster is input-independent -> computed host-side.

Per-core dataflow (all activations kept TRANSPOSED [feature, points] so no
per-layer transposes are needed; W1/W2/W3 natural [K,M] layout == lhsT):
  - x [512,384] loaded, PE-transposed to xT [384,512]
  - AT = W1a.T @ featT + b1  [512,512]  (the feat-dup factorization: the
    first-layer contribution of parent features is shared by all 32 points
    of a parent -> compute once per parent, broadcast-add per k)
  - rel_pack: for each group g of 4 k-values, one matmul with a zero-padded
    W_dec slice puts rel[p, 3k+j] at partition 32s+j (s=k%4) so the K=3
    matmuls below have 32-aligned operand partitions
  - per k (32 iters):
      BT_k [512,512] = W1b.T @ rel_kT     (K=3 matmul)
      h1T = relu(BT_k + AT)               (b1 folded into AT)
      h2T = relu(W2.T @ h1T + b2)
      h3  = relu(h2T.T-slices @ W3 + b3)  (row-major out for clean DMA)
"""

import numpy as np

import concourse.bass as bass
import concourse.tile as tile
from concourse import bacc, mybir
from concourse.bass import ts
from concourse.bass_utils import run_bass_kernel_spmd
from concourse.masks import make_identity

F32 = mybir.dt.float32
F32R = mybir.dt.float32r
BF16 = mybir.dt.bfloat16
AF = mybir.ActivationFunctionType
ALU = mybir.AluOpType

N_PARENTS = 4096
KPTS = 32          # points per parent
NEIGH = 128
FEAT = 256
NCORES = 8
NPL = N_PARENTS // NCORES          # 512 parents per core
PTS = NPL * KPTS                   # 16384 points per core
D1, D2, D3 = 512, 512, 256


def _bcast_ap(ap, n):
    """Prepend a 0-stride dim of size n (partition broadcast for DMA)."""
    return bass.AP(tensor=ap.tensor, offset=ap.offset, ap=[[0, n], *ap.ap])


def build_nc():
    nc = bacc.Bacc("TRN2", target_bir_lowering=False)

    x_d = nc.dram_tensor("x", [NPL, NEIGH + FEAT], F32, kind="ExternalInput")
    wdec_d = nc.dram_tensor("wdec", [NEIGH, KPTS * 3], F32, kind="ExternalInput")
    bdec_d = nc.dram_tensor("bdec", [KPTS * 3], F32, kind="ExternalInput")
    w1_d = nc.dram_tensor("w1", [FEAT + 3, D1], F32, kind="ExternalInput")
    b1_d = nc.dram_tensor("b1", [D1], F32, kind="ExternalInput")
    w2_d = nc.dram_tensor("w2", [D1, D2], F32, kind="ExternalInput")
    b2_d = nc.dram_tensor("b2", [D2], F32, kind="ExternalInput")
    w3_d = nc.dram_tensor("w3", [D2, D3], F32, kind="ExternalInput")
    b3_d = nc.dram_tensor("b3", [D3], F32, kind="ExternalInput")
    rel_d = nc.dram_tensor("rel", [PTS, 3], F32, kind="ExternalOutput")
    h_d = nc.dram_tensor("h", [PTS, D3], F32, kind="ExternalOutput")

    with tile.TileContext(nc) as tc:
        with (
            tc.tile_pool(name="consts", bufs=1) as consts,
            tc.tile_pool(name="work", bufs=3) as work,
            tc.tile_pool(name="ps", bufs=4, space="PSUM") as psp,
            tc.tile_pool(name="psB", bufs=3, space="PSUM") as pspB,
            tc.tile_pool(name="psC", bufs=1, space="PSUM") as pspC,
        ):
            # ---------------- constants / weights ----------------
            ident = consts.tile([128, 128], F32)
            make_identity(nc, ident[:, :])

            x_sb = consts.tile([128, 4, NEIGH + FEAT], F32)   # [p, pc, f]
            nc.sync.dma_start(
                out=x_sb[:, :, :],
                in_=x_d[:, :].rearrange("(pc p) f -> p pc f", p=128),
            )

            wdec_sb = consts.tile([128, KPTS * 3], F32R)
            nc.gpsimd.dma_start(out=wdec_sb[:, :], in_=wdec_d[:, :])

            # bdec staged [3, 32]: bdec32[j, k] = bdec[3k+j] (32 tiny DMAs --
            # partition-strided gather DMAs proved unreliable)
            bdec32 = consts.tile([3, KPTS], F32)
            for k in range(KPTS):
                nc.sync.dma_start(
                    out=bdec32[0:3, k : k + 1],
                    in_=bdec_d[3 * k : 3 * k + 3].rearrange("(j o) -> j o", o=1),
                )
            # bdec broadcast across partitions (free-dim bias for row-major rel)
            bdec_bc = consts.tile([128, KPTS * 3], F32)
            nc.sync.dma_start(out=bdec_bc[:, :], in_=_bcast_ap(bdec_d[:], 128))

            w1a_sb = consts.tile([128, 2, D1], F32R)           # [p, kc, m]
            nc.gpsimd.dma_start(
                out=w1a_sb[:, :, :],
                in_=w1_d[0:FEAT, :].rearrange("(kc p) m -> p kc m", p=128),
            )
            # last-3 rows of W1 in bf16 (the K=3 matmuls run in bf16)
            w1b0 = consts.tile([3, D1], BF16)
            nc.gpsimd.dma_start(out=w1b0[:, :], in_=w1_d[FEAT : FEAT + 3, :])
            w2_sb = consts.tile([128, 4, D2], BF16)
            nc.gpsimd.dma_start(
                out=w2_sb[:, :, :],
                in_=w2_d[:, :].rearrange("(kc p) m -> p kc m", p=128),
            )
            w3_sb = consts.tile([128, 4, D3], BF16)
            nc.gpsimd.dma_start(
                out=w3_sb[:, :, :],
                in_=w3_d[:, :].rearrange("(kc p) m -> p kc m", p=128),
            )
            b1_sb = consts.tile([128, 4], F32)
            nc.sync.dma_start(
                out=b1_sb[:, :], in_=b1_d[:].rearrange("(mc p) -> p mc", p=128)
            )
            b2_sb = consts.tile([128, 4], F32)
            nc.sync.dma_start(
                out=b2_sb[:, :], in_=b2_d[:].rearrange("(mc p) -> p mc", p=128)
            )

            # ---------------- transpose input: xT [384, 512] ----------------
            xT = consts.tile([128, 3, NPL], F32R)              # [q, fc, p]
            for pc in range(4):
                for fc in range(3):
                    pt = psp.tile([128, 128], F32, tag="ps")
                    nc.tensor.transpose(
                        pt[:, :], x_sb[:, pc, ts(fc, 128)], ident[:, :]
                    )
                    nc.vector.tensor_copy(out=xT[:, fc, ts(pc, 128)], in_=pt[:, :])

            # bf16 copies for the rel_k production matmuls
            wdec_bf = consts.tile([128, KPTS * 3], BF16)
            nc.vector.tensor_copy(out=wdec_bf[:, :], in_=wdec_sb[:, :])
            neigh_bf = consts.tile([128, NPL], BF16)
            nc.vector.tensor_copy(out=neigh_bf[:, :], in_=xT[:, 0, :])

            # ---------------- rel_k slabs [3, 32, 512] in bf16 ----------------
            # rel_k[j, k, p] = rel[p, 3k+j] + bdec[3k+j]; lhsT is a 3-column
            # slice of wdec -> output lands at partitions 0:2, M=3
            rel_k = consts.tile([3, KPTS, NPL], BF16)
            for k in range(KPTS):
                pt = pspC.tile([3, NPL], F32, tag="psC")
                nc.tensor.matmul(
                    pt[:, :],
                    wdec_bf[:, 3 * k : 3 * k + 3],
                    neigh_bf[:, :],
                )
                nc.vector.tensor_scalar_add(
                    out=rel_k[:, k, :], in0=pt[:, :], scalar1=bdec32[:, k : k + 1]
                )

            # ---------------- rel row-major output ----------------
            rel_rm = consts.tile([128, 4, KPTS * 3], F32)     # [p, pc, (k j)]
            rel_view = rel_d[:, :].rearrange("(p a) b -> p (a b)", a=KPTS)
            for pc in range(4):
                pt = psp.tile([128, KPTS * 3], F32, tag="ps")
                nc.tensor.matmul(
                    pt[:, :],
                    xT[:, 0, ts(pc, 128)],
                    wdec_sb[:, :],
                )
                nc.vector.tensor_tensor(
                    out=rel_rm[:, pc, :], in0=pt[:, :], in1=bdec_bc[:, :], op=ALU.add
                )
                nc.sync.dma_start(out=rel_view[ts(pc, 128), :], in_=rel_rm[:, pc, :])

            # ---------------- AT = W1a.T @ featT + b1  [512, 512] ----------------
            at_b = consts.tile([128, 4, NPL], F32)           # [m, mc, p]
            for mc in range(4):
                pt = psp.tile([128, NPL], F32, tag="ps")
                for kc in range(2):
                    nc.tensor.matmul(
                        pt[:, :],
                        w1a_sb[:, kc, ts(mc, 128)],
                        xT[:, 1 + kc, :],
                        start=(kc == 0),
                        stop=(kc == 1),
                    )
                nc.vector.tensor_scalar_add(
                    out=at_b[:, mc, :], in0=pt[:, :], scalar1=b1_sb[:, mc : mc + 1]
                )

            # ones/b3 rows in bf16 for the L3 bias-injection matmul
            ones_r = consts.tile([1, 128], BF16)
            ones_f = consts.tile([1, 128], F32)
            nc.vector.memset(ones_f[:, :], 1.0)
            nc.vector.tensor_copy(out=ones_r[:, :], in_=ones_f[:, :])
            b3_row = consts.tile([1, D3], BF16)
            nc.gpsimd.dma_start(
                out=b3_row[:, :], in_=b3_d[:].rearrange("(o d) -> o d", o=1)
            )

            # ---------------- main loop over k ----------------
            h_view = h_d[:, :].rearrange("(p k) d -> k p d", k=KPTS)
            for k in range(KPTS):
                # h1T_k = relu(AT + W1b.T @ rel_kT)  [512, 512]
                h1 = work.tile([128, 4, NPL], BF16, tag="h1")
                for mc in range(4):
                    pt = psp.tile([128, NPL], F32, tag="ps")
                    nc.tensor.matmul(
                        pt[:, :],
                        w1b0[:, ts(mc, 128)],
                        rel_k[:, k, :],
                    )
                    nc.vector.tensor_tensor(
                        out=pt[:, :], in0=pt[:, :], in1=at_b[:, mc, :], op=ALU.add
                    )
                    nc.scalar.activation(
                        out=h1[:, mc, :], in_=pt[:, :], func=AF.Relu
                    )

                # h2T_k = relu(W2.T @ h1T + b2)  [512, 512]
                h2 = work.tile([128, 4, NPL], BF16, tag="h2")
                for mc in range(4):
                    pt = pspB.tile([128, NPL], F32, tag="psB")
                    for kc in range(4):
                        nc.tensor.matmul(
                            pt[:, :],
                            w2_sb[:, kc, ts(mc, 128)],
                            h1[:, kc, :],
                            start=(kc == 0),
                            stop=(kc == 3),
                        )
                    nc.scalar.activation(
                        out=h2[:, mc, :],
                        in_=pt[:, :],
                        func=AF.Relu,
                        bias=b2_sb[:, mc : mc + 1],
                    )

                # h3_k = relu(h2T.T @ W3 + b3) row-major [512, 256]
                # b3 is injected via a K=1 ones-row matmul (bias varies along
                # the free dim here, which ACT bias cannot express)
                h3 = work.tile([128, 4, D3], F32, tag="h3")
                for rc in range(4):
                    pt = pspC.tile([128, D3], F32, tag="psC")
                    nc.tensor.matmul(
                        pt[:, :], ones_r[:, :], b3_row[:, :],
                        start=True, stop=False,
                    )
                    for kc in range(4):
                        nc.tensor.matmul(
                            pt[:, :],
                            h2[:, kc, ts(rc, 128)],
                            w3_sb[:, kc, :],
                            start=False,
                            stop=(kc == 3),
                        )
                    nc.vector.tensor_relu(out=h3[:, rc, :], in_=pt[:, :])
                    nc.sync.dma_start(
                        out=h_view[k, ts(rc, 128), :], in_=h3[:, rc, :]
                    )

    nc.compile()
    return nc


_NC_CACHE = None


def _get_nc():
    global _NC_CACHE
    if _NC_CACHE is None:
        _NC_CACHE = build_nc()
    return _NC_CACHE


def kernel(input_features, W_dec, b_dec, W1, b1, W2, b2, W3, b3):
    x = np.ascontiguousarray(np.asarray(input_features, dtype=np.float32))
    args = {
        "wdec": np.ascontiguousarray(np.asarray(W_dec, dtype=np.float32)),
        "bdec": np.ascontiguousarray(np.asarray(b_dec, dtype=np.float32)),
        "w1": np.ascontiguousarray(np.asarray(W1, dtype=np.float32)),
        "b1": np.ascontiguousarray(np.asarray(b1, dtype=np.float32)),
        "w2": np.ascontiguousarray(np.asarray(W2, dtype=np.float32)),
        "b2": np.ascontiguousarray(np.asarray(b2, dtype=np.float32)),
        "w3": np.ascontiguousarray(np.asarray(W3, dtype=np.float32)),
        "b3": np.ascontiguousarray(np.asarray(b3, dtype=np.float32)),
    }
    nc = _get_nc()
    in_maps = [
        {"x": np.ascontiguousarray(x[c * NPL : (c + 1) * NPL]), **args}
        for c in range(NCORES)
    ]
    res = run_bass_kernel_spmd(nc, in_maps, list(range(NCORES)))
    rel = np.concatenate([res.results[c]["rel"] for c in range(NCORES)], axis=0)
    h = np.concatenate([res.results[c]["h"] for c in range(NCORES)], axis=0)
    cluster = np.repeat(np.arange(N_PARENTS, dtype=np.int32), KPTS)
    return rel, h, cluster


# revision 30
# speedup vs baseline: 1.0232x; 1.0232x over previous
"""Trainium2 Bass kernel for nn_MiddleLayerDecoderSplit.

Reference computation (per parent cluster p of N=4096, K=32 points):
    rel = x[:, :128] @ W_dec + b_dec            # [N, 96] -> [N*K, 3]
    h0  = concat(feat[cluster], rel_pts)        # [N*K, 259]
    h   = relu(relu(relu(h0@W1+b1)@W2+b2)@W3+b3)  # [N*K, 256]
Outputs: (rel_pts [N*K,3], h [N*K,256], cluster [N*K] int32)

Sharding: data-parallel over parent rows, 512 parents per core x 8 cores.
Weights replicated. cluster is input-independent -> computed host-side.

Per-core dataflow (all activations kept TRANSPOSED [feature, points] so no
per-layer transposes are needed; W1/W2/W3 natural [K,M] layout == lhsT):
  - x [512,384] loaded, PE-transposed to xT [384,512]
  - AT = W1a.T @ featT + b1  [512,512]  (the feat-dup factorization: the
    first-layer contribution of parent features is shared by all 32 points
    of a parent -> compute once per parent, broadcast-add per k)
  - rel_pack: for each group g of 4 k-values, one matmul with a zero-padded
    W_dec slice puts rel[p, 3k+j] at partition 32s+j (s=k%4) so the K=3
    matmuls below have 32-aligned operand partitions
  - per k (32 iters):
      BT_k [512,512] = W1b.T @ rel_kT     (K=3 matmul)
      h1T = relu(BT_k + AT)               (b1 folded into AT)
      h2T = relu(W2.T @ h1T + b2)
      h3  = relu(h2T.T-slices @ W3 + b3)  (row-major out for clean DMA)
"""

import numpy as np

import concourse.bass as bass
import concourse.tile as tile
from concourse import bacc, mybir
from concourse.bass import ts
from concourse.bass_utils import run_bass_kernel_spmd
from concourse.masks import make_identity

F32 = mybir.dt.float32
F32R = mybir.dt.float32r
BF16 = mybir.dt.bfloat16
AF = mybir.ActivationFunctionType
ALU = mybir.AluOpType

N_PARENTS = 4096
KPTS = 32          # points per parent
NEIGH = 128
FEAT = 256
NCORES = 8
NPL = N_PARENTS // NCORES          # 512 parents per core
PTS = NPL * KPTS                   # 16384 points per core
D1, D2, D3 = 512, 512, 256


def _bcast_ap(ap, n):
    """Prepend a 0-stride dim of size n (partition broadcast for DMA)."""
    return bass.AP(tensor=ap.tensor, offset=ap.offset, ap=[[0, n], *ap.ap])


def build_nc():
    nc = bacc.Bacc("TRN2", target_bir_lowering=False)

    x_d = nc.dram_tensor("x", [NPL, NEIGH + FEAT], F32, kind="ExternalInput")
    wdec_d = nc.dram_tensor("wdec", [NEIGH, KPTS * 3], F32, kind="ExternalInput")
    bdec_d = nc.dram_tensor("bdec", [KPTS * 3], F32, kind="ExternalInput")
    w1_d = nc.dram_tensor("w1", [FEAT + 3, D1], F32, kind="ExternalInput")
    b1_d = nc.dram_tensor("b1", [D1], F32, kind="ExternalInput")
    w2_d = nc.dram_tensor("w2", [D1, D2], F32, kind="ExternalInput")
    b2_d = nc.dram_tensor("b2", [D2], F32, kind="ExternalInput")
    w3_d = nc.dram_tensor("w3", [D2, D3], F32, kind="ExternalInput")
    b3_d = nc.dram_tensor("b3", [D3], F32, kind="ExternalInput")
    rel_d = nc.dram_tensor("rel", [PTS, 3], F32, kind="ExternalOutput")
    h_d = nc.dram_tensor("h", [PTS, D3], F32, kind="ExternalOutput")

    with tile.TileContext(nc) as tc:
        with (
            tc.tile_pool(name="consts", bufs=1) as consts,
            tc.tile_pool(name="work", bufs=3) as work,
            tc.tile_pool(name="ps", bufs=4, space="PSUM") as psp,
            tc.tile_pool(name="psB", bufs=2, space="PSUM") as pspB,
            tc.tile_pool(name="psC", bufs=2, space="PSUM") as pspC,
        ):
            # ---------------- constants / weights ----------------
            ident = consts.tile([128, 128], F32)
            make_identity(nc, ident[:, :])

            x_sb = consts.tile([128, 4, NEIGH + FEAT], F32)   # [p, pc, f]
            nc.sync.dma_start(
                out=x_sb[:, :, :],
                in_=x_d[:, :].rearrange("(pc p) f -> p pc f", p=128),
            )

            wdec_sb = consts.tile([128, KPTS * 3], F32R)
            nc.gpsimd.dma_start(out=wdec_sb[:, :], in_=wdec_d[:, :])

            # bdec staged [3, 32]: bdec32[j, k] = bdec[3k+j] (32 tiny DMAs --
            # partition-strided gather DMAs proved unreliable)
            bdec32 = consts.tile([3, KPTS], F32)
            for k in range(KPTS):
                nc.sync.dma_start(
                    out=bdec32[0:3, k : k + 1],
                    in_=bdec_d[3 * k : 3 * k + 3].rearrange("(j o) -> j o", o=1),
                )
            # bdec broadcast across partitions (free-dim bias for row-major rel)
            bdec_bc = consts.tile([128, KPTS * 3], F32)
            nc.sync.dma_start(out=bdec_bc[:, :], in_=_bcast_ap(bdec_d[:], 128))

            w1a_sb = consts.tile([128, 2, D1], F32R)           # [p, kc, m]
            nc.gpsimd.dma_start(
                out=w1a_sb[:, :, :],
                in_=w1_d[0:FEAT, :].rearrange("(kc p) m -> p kc m", p=128),
            )
            # last-3 rows of W1 in bf16 (the K=3 matmuls run in bf16)
            w1b0 = consts.tile([3, D1], BF16)
            nc.gpsimd.dma_start(out=w1b0[:, :], in_=w1_d[FEAT : FEAT + 3, :])
            w2_sb = consts.tile([128, 4, D2], BF16)
            nc.gpsimd.dma_start(
                out=w2_sb[:, :, :],
                in_=w2_d[:, :].rearrange("(kc p) m -> p kc m", p=128),
            )
            w3_sb = consts.tile([128, 4, D3], BF16)
            nc.gpsimd.dma_start(
                out=w3_sb[:, :, :],
                in_=w3_d[:, :].rearrange("(kc p) m -> p kc m", p=128),
            )
            b1_sb = consts.tile([128, 4], F32)
            nc.sync.dma_start(
                out=b1_sb[:, :], in_=b1_d[:].rearrange("(mc p) -> p mc", p=128)
            )
            b2_sb = consts.tile([128, 4], F32)
            nc.sync.dma_start(
                out=b2_sb[:, :], in_=b2_d[:].rearrange("(mc p) -> p mc", p=128)
            )

            # ---------------- transpose input: xT [384, 512] ----------------
            xT = consts.tile([128, 3, NPL], F32R)              # [q, fc, p]
            for pc in range(4):
                for fc in range(3):
                    pt = psp.tile([128, 128], F32, tag="ps")
                    nc.tensor.transpose(
                        pt[:, :], x_sb[:, pc, ts(fc, 128)], ident[:, :]
                    )
                    nc.vector.tensor_copy(out=xT[:, fc, ts(pc, 128)], in_=pt[:, :])

            # bf16 copies for the rel_k production matmuls
            wdec_bf = consts.tile([128, KPTS * 3], BF16)
            nc.vector.tensor_copy(out=wdec_bf[:, :], in_=wdec_sb[:, :])
            neigh_bf = consts.tile([128, NPL], BF16)
            nc.vector.tensor_copy(out=neigh_bf[:, :], in_=xT[:, 0, :])

            # ---------------- rel_k slabs [3, 32, 512] in bf16 ----------------
            # rel_k[j, k, p] = rel[p, 3k+j] + bdec[3k+j]; lhsT is a 3-column
            # slice of wdec -> output lands at partitions 0:2, M=3
            rel_k = consts.tile([3, KPTS, NPL], BF16)
            for k in range(KPTS):
                pt = pspC.tile([3, NPL], F32, tag="psC")
                nc.tensor.matmul(
                    pt[:, :],
                    wdec_bf[:, 3 * k : 3 * k + 3],
                    neigh_bf[:, :],
                )
                nc.vector.tensor_scalar_add(
                    out=rel_k[:, k, :], in0=pt[:, :], scalar1=bdec32[:, k : k + 1]
                )

            # ---------------- rel row-major output ----------------
            rel_rm = consts.tile([128, 4, KPTS * 3], F32)     # [p, pc, (k j)]
            rel_view = rel_d[:, :].rearrange("(p a) b -> p (a b)", a=KPTS)
            for pc in range(4):
                pt = psp.tile([128, KPTS * 3], F32, tag="ps")
                nc.tensor.matmul(
                    pt[:, :],
                    xT[:, 0, ts(pc, 128)],
                    wdec_sb[:, :],
                )
                nc.vector.tensor_tensor(
                    out=rel_rm[:, pc, :], in0=pt[:, :], in1=bdec_bc[:, :], op=ALU.add
                )
                nc.sync.dma_start(out=rel_view[ts(pc, 128), :], in_=rel_rm[:, pc, :])

            # ---------------- AT = W1a.T @ featT + b1  [512, 512] ----------------
            at_b = consts.tile([128, 4, NPL], F32)           # [m, mc, p]
            for mc in range(4):
                pt = psp.tile([128, NPL], F32, tag="ps")
                for kc in range(2):
                    nc.tensor.matmul(
                        pt[:, :],
                        w1a_sb[:, kc, ts(mc, 128)],
                        xT[:, 1 + kc, :],
                        start=(kc == 0),
                        stop=(kc == 1),
                    )
                nc.vector.tensor_scalar_add(
                    out=at_b[:, mc, :], in0=pt[:, :], scalar1=b1_sb[:, mc : mc + 1]
                )

            # ones/b3 rows in bf16 for the L3 bias-injection matmul
            ones_r = consts.tile([1, 128], BF16)
            ones_f = consts.tile([1, 128], F32)
            nc.vector.memset(ones_f[:, :], 1.0)
            nc.vector.tensor_copy(out=ones_r[:, :], in_=ones_f[:, :])
            b3_row = consts.tile([1, D3], BF16)
            nc.gpsimd.dma_start(
                out=b3_row[:, :], in_=b3_d[:].rearrange("(o d) -> o d", o=1)
            )

            # ---------------- main loop over k ----------------
            # software-pipelined emission: L1 for k+1 is emitted ahead of
            # L2/L3 for k so the PE always has ready high-priority work at
            # the stage boundaries
            h_view = h_d[:, :].rearrange("(p k) d -> k p d", k=KPTS)

            def emit_l1(k):
                h1 = work.tile([128, 4, NPL], BF16, tag="h1")
                for mc in range(4):
                    pt = psp.tile([128, NPL], F32, tag="ps")
                    nc.tensor.matmul(
                        pt[:, :],
                        w1b0[:, ts(mc, 128)],
                        rel_k[:, k, :],
                    )
                    nc.vector.tensor_tensor(
                        out=pt[:, :], in0=pt[:, :], in1=at_b[:, mc, :], op=ALU.add
                    )
                    nc.scalar.activation(
                        out=h1[:, mc, :], in_=pt[:, :], func=AF.Relu
                    )
                return h1

            h1 = emit_l1(0)
            for k in range(KPTS):
                h1_next = emit_l1(k + 1) if k + 1 < KPTS else None

                # h2T_k = relu(W2.T @ h1T + b2)  [512, 512]
                h2 = work.tile([128, 4, NPL], BF16, tag="h2")
                for mc in range(4):
                    pt = pspB.tile([128, NPL], F32, tag="psB")
                    for kc in range(4):
                        nc.tensor.matmul(
                            pt[:, :],
                            w2_sb[:, kc, ts(mc, 128)],
                            h1[:, kc, :],
                            start=(kc == 0),
                            stop=(kc == 3),
                        )
                    nc.scalar.activation(
                        out=h2[:, mc, :],
                        in_=pt[:, :],
                        func=AF.Relu,
                        bias=b2_sb[:, mc : mc + 1],
                    )

                # h3_k = relu(h2T.T @ W3 + b3) row-major [512, 256]
                # b3 is injected via a K=1 ones-row matmul (bias varies along
                # the free dim here, which ACT bias cannot express)
                h3 = work.tile([128, 4, D3], F32, tag="h3")
                for rc in range(4):
                    pt = pspC.tile([128, D3], F32, tag="psC")
                    nc.tensor.matmul(
                        pt[:, :], ones_r[:, :], b3_row[:, :],
                        start=True, stop=False,
                    )
                    for kc in range(4):
                        nc.tensor.matmul(
                            pt[:, :],
                            h2[:, kc, ts(rc, 128)],
                            w3_sb[:, kc, :],
                            start=False,
                            stop=(kc == 3),
                        )
                    nc.vector.tensor_relu(out=h3[:, rc, :], in_=pt[:, :])
                    nc.sync.dma_start(
                        out=h_view[k, ts(rc, 128), :], in_=h3[:, rc, :]
                    )

                h1 = h1_next

    nc.compile()
    return nc


_NC_CACHE = None


def _get_nc():
    global _NC_CACHE
    if _NC_CACHE is None:
        _NC_CACHE = build_nc()
    return _NC_CACHE


def kernel(input_features, W_dec, b_dec, W1, b1, W2, b2, W3, b3):
    x = np.ascontiguousarray(np.asarray(input_features, dtype=np.float32))
    args = {
        "wdec": np.ascontiguousarray(np.asarray(W_dec, dtype=np.float32)),
        "bdec": np.ascontiguousarray(np.asarray(b_dec, dtype=np.float32)),
        "w1": np.ascontiguousarray(np.asarray(W1, dtype=np.float32)),
        "b1": np.ascontiguousarray(np.asarray(b1, dtype=np.float32)),
        "w2": np.ascontiguousarray(np.asarray(W2, dtype=np.float32)),
        "b2": np.ascontiguousarray(np.asarray(b2, dtype=np.float32)),
        "w3": np.ascontiguousarray(np.asarray(W3, dtype=np.float32)),
        "b3": np.ascontiguousarray(np.asarray(b3, dtype=np.float32)),
    }
    nc = _get_nc()
    in_maps = [
        {"x": np.ascontiguousarray(x[c * NPL : (c + 1) * NPL]), **args}
        for c in range(NCORES)
    ]
    res = run_bass_kernel_spmd(nc, in_maps, list(range(NCORES)))
    rel = np.concatenate([res.results[c]["rel"] for c in range(NCORES)], axis=0)
    h = np.concatenate([res.results[c]["h"] for c in range(NCORES)], axis=0)
    cluster = np.repeat(np.arange(N_PARENTS, dtype=np.int32), KPTS)
    return rel, h, cluster


# revision 34
# speedup vs baseline: 1.0462x; 1.0225x over previous
"""Trainium2 Bass kernel for nn_MiddleLayerDecoderSplit.

Reference computation (per parent cluster p of N=4096, K=32 points):
    rel = x[:, :128] @ W_dec + b_dec            # [N, 96] -> [N*K, 3]
    h0  = concat(feat[cluster], rel_pts)        # [N*K, 259]
    h   = relu(relu(relu(h0@W1+b1)@W2+b2)@W3+b3)  # [N*K, 256]
Outputs: (rel_pts [N*K,3], h [N*K,256], cluster [N*K] int32)

Sharding: data-parallel over parent rows, 512 parents per core x 8 cores.
Weights replicated. cluster is input-independent -> computed host-side.

Per-core dataflow (all activations kept TRANSPOSED [feature, points] so no
per-layer transposes are needed; W1/W2/W3 natural [K,M] layout == lhsT):
  - x [512,384] loaded, PE-transposed to xT [384,512]
  - AT = W1a.T @ featT + b1  [512,512]  (the feat-dup factorization: the
    first-layer contribution of parent features is shared by all 32 points
    of a parent -> compute once per parent, broadcast-add per k)
  - rel_k [3,32,512] bf16: per k, one matmul with a 3-column slice of W_dec
    as the stationary puts rel_kT at partitions 0:2 (+ b_dec bias on evac)
  - per k (32 iters):
      psum = W1b.T @ rel_kT  (K=3 bf16 matmul)
      h1T  = relu(psum + AT)              (DVE add, ACT relu; b1 in AT)
      h2T  = relu(W2.T @ h1T + b2)        (bf16 weights/activations)
      h3   = relu(h2T.T-slices @ W3 + b3) (row-major; b3 injected via a
                                           K=1 ones-row matmul)
The h-chain matmuls run in bf16 (1 cycle/row vs ~2 for fp32r); the rel
output path stays fp32r for full precision. exec ~299us on 8 cores.
"""

import time

import numpy as np

import concourse.bass as bass
import concourse.tile as tile
from concourse import bacc, mybir
from concourse.bass import ts
from concourse.bass_utils import run_bass_kernel_spmd
from concourse.masks import make_identity

F32 = mybir.dt.float32
F32R = mybir.dt.float32r
BF16 = mybir.dt.bfloat16
AF = mybir.ActivationFunctionType
ALU = mybir.AluOpType

N_PARENTS = 4096
KPTS = 32          # points per parent
NEIGH = 128
FEAT = 256
NCORES = 8
NPL = N_PARENTS // NCORES          # 512 parents per core
PTS = NPL * KPTS                   # 16384 points per core
D1, D2, D3 = 512, 512, 256


def _bcast_ap(ap, n):
    """Prepend a 0-stride dim of size n (partition broadcast for DMA)."""
    return bass.AP(tensor=ap.tensor, offset=ap.offset, ap=[[0, n], *ap.ap])


def build_nc():
    nc = bacc.Bacc("TRN2", target_bir_lowering=False)

    x_d = nc.dram_tensor("x", [NPL, NEIGH + FEAT], F32, kind="ExternalInput")
    wdec_d = nc.dram_tensor("wdec", [NEIGH, KPTS * 3], F32, kind="ExternalInput")
    bdec_d = nc.dram_tensor("bdec", [KPTS * 3], F32, kind="ExternalInput")
    w1_d = nc.dram_tensor("w1", [FEAT + 3, D1], F32, kind="ExternalInput")
    b1_d = nc.dram_tensor("b1", [D1], F32, kind="ExternalInput")
    w2_d = nc.dram_tensor("w2", [D1, D2], F32, kind="ExternalInput")
    b2_d = nc.dram_tensor("b2", [D2], F32, kind="ExternalInput")
    w3_d = nc.dram_tensor("w3", [D2, D3], F32, kind="ExternalInput")
    b3_d = nc.dram_tensor("b3", [D3], F32, kind="ExternalInput")
    rel_d = nc.dram_tensor("rel", [PTS, 3], F32, kind="ExternalOutput")
    h_d = nc.dram_tensor("h", [PTS, D3], F32, kind="ExternalOutput")

    with tile.TileContext(nc) as tc:
        with (
            tc.tile_pool(name="consts", bufs=1) as consts,
            tc.tile_pool(name="work", bufs=3) as work,
            tc.tile_pool(name="ps", bufs=4, space="PSUM") as psp,
            tc.tile_pool(name="psB", bufs=2, space="PSUM") as pspB,
            tc.tile_pool(name="psC", bufs=2, space="PSUM") as pspC,
        ):
            # ---------------- constants / weights ----------------
            ident = consts.tile([128, 128], F32)
            make_identity(nc, ident[:, :])

            x_sb = consts.tile([128, 4, NEIGH + FEAT], F32)   # [p, pc, f]
            nc.sync.dma_start(
                out=x_sb[:, :, :],
                in_=x_d[:, :].rearrange("(pc p) f -> p pc f", p=128),
            )

            wdec_sb = consts.tile([128, KPTS * 3], F32R)
            nc.gpsimd.dma_start(out=wdec_sb[:, :], in_=wdec_d[:, :])

            # bdec staged [3, 32]: bdec32[j, k] = bdec[3k+j] (32 tiny DMAs --
            # partition-strided gather DMAs proved unreliable)
            bdec32 = consts.tile([3, KPTS], F32)
            for k in range(KPTS):
                nc.sync.dma_start(
                    out=bdec32[0:3, k : k + 1],
                    in_=bdec_d[3 * k : 3 * k + 3].rearrange("(j o) -> j o", o=1),
                )
            # bdec broadcast across partitions (free-dim bias for row-major rel)
            bdec_bc = consts.tile([128, KPTS * 3], F32)
            nc.sync.dma_start(out=bdec_bc[:, :], in_=_bcast_ap(bdec_d[:], 128))

            w1a_sb = consts.tile([128, 2, D1], F32R)           # [p, kc, m]
            nc.gpsimd.dma_start(
                out=w1a_sb[:, :, :],
                in_=w1_d[0:FEAT, :].rearrange("(kc p) m -> p kc m", p=128),
            )
            # last-3 rows of W1 in bf16 (the K=3 matmuls run in bf16)
            w1b0 = consts.tile([3, D1], BF16)
            nc.gpsimd.dma_start(out=w1b0[:, :], in_=w1_d[FEAT : FEAT + 3, :])
            w2_sb = consts.tile([128, 4, D2], BF16)
            nc.gpsimd.dma_start(
                out=w2_sb[:, :, :],
                in_=w2_d[:, :].rearrange("(kc p) m -> p kc m", p=128),
            )
            w3_sb = consts.tile([128, 4, D3], BF16)
            nc.gpsimd.dma_start(
                out=w3_sb[:, :, :],
                in_=w3_d[:, :].rearrange("(kc p) m -> p kc m", p=128),
            )
            b1_sb = consts.tile([128, 4], F32)
            nc.sync.dma_start(
                out=b1_sb[:, :], in_=b1_d[:].rearrange("(mc p) -> p mc", p=128)
            )
            b2_sb = consts.tile([128, 4], F32)
            nc.sync.dma_start(
                out=b2_sb[:, :], in_=b2_d[:].rearrange("(mc p) -> p mc", p=128)
            )

            # ---------------- transpose input: xT [384, 512] ----------------
            xT = consts.tile([128, 3, NPL], F32R)              # [q, fc, p]
            for pc in range(4):
                for fc in range(3):
                    pt = psp.tile([128, 128], F32, tag="ps")
                    nc.tensor.transpose(
                        pt[:, :], x_sb[:, pc, ts(fc, 128)], ident[:, :]
                    )
                    nc.vector.tensor_copy(out=xT[:, fc, ts(pc, 128)], in_=pt[:, :])

            # bf16 copies for the rel_k production matmuls
            wdec_bf = consts.tile([128, KPTS * 3], BF16)
            nc.vector.tensor_copy(out=wdec_bf[:, :], in_=wdec_sb[:, :])
            neigh_bf = consts.tile([128, NPL], BF16)
            nc.vector.tensor_copy(out=neigh_bf[:, :], in_=xT[:, 0, :])

            # ---------------- rel_k slabs [3, 32, 512] in bf16 ----------------
            # rel_k[j, k, p] = rel[p, 3k+j] + bdec[3k+j]; lhsT is a 3-column
            # slice of wdec -> output lands at partitions 0:2, M=3
            rel_k = consts.tile([3, KPTS, NPL], BF16)
            for k in range(KPTS):
                pt = pspC.tile([3, NPL], F32, tag="psC")
                nc.tensor.matmul(
                    pt[:, :],
                    wdec_bf[:, 3 * k : 3 * k + 3],
                    neigh_bf[:, :],
                )
                nc.vector.tensor_scalar_add(
                    out=rel_k[:, k, :], in0=pt[:, :], scalar1=bdec32[:, k : k + 1]
                )

            # ---------------- rel row-major output ----------------
            rel_rm = consts.tile([128, 4, KPTS * 3], F32)     # [p, pc, (k j)]
            rel_view = rel_d[:, :].rearrange("(p a) b -> p (a b)", a=KPTS)
            for pc in range(4):
                pt = psp.tile([128, KPTS * 3], F32, tag="ps")
                nc.tensor.matmul(
                    pt[:, :],
                    xT[:, 0, ts(pc, 128)],
                    wdec_sb[:, :],
                )
                nc.vector.tensor_tensor(
                    out=rel_rm[:, pc, :], in0=pt[:, :], in1=bdec_bc[:, :], op=ALU.add
                )
                nc.sync.dma_start(out=rel_view[ts(pc, 128), :], in_=rel_rm[:, pc, :])

            # ---------------- AT = W1a.T @ featT + b1  [512, 512] ----------------
            at_b = consts.tile([128, 4, NPL], F32)           # [m, mc, p]
            for mc in range(4):
                pt = psp.tile([128, NPL], F32, tag="ps")
                for kc in range(2):
                    nc.tensor.matmul(
                        pt[:, :],
                        w1a_sb[:, kc, ts(mc, 128)],
                        xT[:, 1 + kc, :],
                        start=(kc == 0),
                        stop=(kc == 1),
                    )
                nc.vector.tensor_scalar_add(
                    out=at_b[:, mc, :], in0=pt[:, :], scalar1=b1_sb[:, mc : mc + 1]
                )

            # ones/b3 rows in bf16 for the L3 bias-injection matmul
            ones_r = consts.tile([1, 128], BF16)
            ones_f = consts.tile([1, 128], F32)
            nc.vector.memset(ones_f[:, :], 1.0)
            nc.vector.tensor_copy(out=ones_r[:, :], in_=ones_f[:, :])
            b3_row = consts.tile([1, D3], BF16)
            nc.gpsimd.dma_start(
                out=b3_row[:, :], in_=b3_d[:].rearrange("(o d) -> o d", o=1)
            )

            # ---------------- main loop over k ----------------
            h_view = h_d[:, :].rearrange("(p k) d -> k p d", k=KPTS)
            for k in range(KPTS):
                # h1T_k = relu(AT + W1b.T @ rel_kT)  [512, 512]
                h1 = work.tile([128, 4, NPL], BF16, tag="h1")
                for mc in range(4):
                    pt = psp.tile([128, NPL], F32, tag="ps")
                    nc.tensor.matmul(
                        pt[:, :],
                        w1b0[:, ts(mc, 128)],
                        rel_k[:, k, :],
                    )
                    nc.vector.tensor_tensor(
                        out=pt[:, :], in0=pt[:, :], in1=at_b[:, mc, :], op=ALU.add
                    )
                    nc.scalar.activation(
                        out=h1[:, mc, :], in_=pt[:, :], func=AF.Relu
                    )

                # h2T_k = relu(W2.T @ h1T + b2)  [512, 512]
                h2 = work.tile([128, 4, NPL], BF16, tag="h2")
                for mc in range(4):
                    pt = pspB.tile([128, NPL], F32, tag="psB")
                    for kc in range(4):
                        nc.tensor.matmul(
                            pt[:, :],
                            w2_sb[:, kc, ts(mc, 128)],
                            h1[:, kc, :],
                            start=(kc == 0),
                            stop=(kc == 3),
                        )
                    nc.scalar.activation(
                        out=h2[:, mc, :],
                        in_=pt[:, :],
                        func=AF.Relu,
                        bias=b2_sb[:, mc : mc + 1],
                    )

                # h3_k = relu(h2T.T @ W3 + b3) row-major [512, 256]
                # b3 is injected via a K=1 ones-row matmul (bias varies along
                # the free dim here, which ACT bias cannot express)
                h3 = work.tile([128, 4, D3], F32, tag="h3")
                for rc in range(4):
                    pt = pspC.tile([128, D3], F32, tag="psC")
                    nc.tensor.matmul(
                        pt[:, :], ones_r[:, :], b3_row[:, :],
                        start=True, stop=False,
                    )
                    for kc in range(4):
                        nc.tensor.matmul(
                            pt[:, :],
                            h2[:, kc, ts(rc, 128)],
                            w3_sb[:, kc, :],
                            start=False,
                            stop=(kc == 3),
                        )
                    nc.vector.tensor_relu(out=h3[:, rc, :], in_=pt[:, :])
                    nc.sync.dma_start(
                        out=h_view[k, ts(rc, 128), :], in_=h3[:, rc, :]
                    )

    nc.compile()
    return nc


_NC_CACHE = None


def _get_nc():
    global _NC_CACHE
    if _NC_CACHE is None:
        _NC_CACHE = build_nc()
    return _NC_CACHE


def kernel(input_features, W_dec, b_dec, W1, b1, W2, b2, W3, b3):
    x = np.ascontiguousarray(np.asarray(input_features, dtype=np.float32))
    args = {
        "wdec": np.ascontiguousarray(np.asarray(W_dec, dtype=np.float32)),
        "bdec": np.ascontiguousarray(np.asarray(b_dec, dtype=np.float32)),
        "w1": np.ascontiguousarray(np.asarray(W1, dtype=np.float32)),
        "b1": np.ascontiguousarray(np.asarray(b1, dtype=np.float32)),
        "w2": np.ascontiguousarray(np.asarray(W2, dtype=np.float32)),
        "b2": np.ascontiguousarray(np.asarray(b2, dtype=np.float32)),
        "w3": np.ascontiguousarray(np.asarray(W3, dtype=np.float32)),
        "b3": np.ascontiguousarray(np.asarray(b3, dtype=np.float32)),
    }
    nc = _get_nc()
    in_maps = [
        {"x": np.ascontiguousarray(x[c * NPL : (c + 1) * NPL]), **args}
        for c in range(NCORES)
    ]
    # the first execute after a profiled session occasionally finds the
    # device wedged (NRT_EXEC_UNIT_UNRECOVERABLE); it recovers on retry
    for attempt in range(3):
        try:
            res = run_bass_kernel_spmd(nc, in_maps, list(range(NCORES)))
            break
        except Exception:
            if attempt == 2:
                raise
            time.sleep(5)
    rel = np.concatenate([res.results[c]["rel"] for c in range(NCORES)], axis=0)
    h = np.concatenate([res.results[c]["h"] for c in range(NCORES)], axis=0)
    cluster = np.repeat(np.arange(N_PARENTS, dtype=np.int32), KPTS)
    return rel, h, cluster


# revision 35
# speedup vs baseline: 1.0503x; 1.0039x over previous
"""Trainium2 Bass kernel for nn_MiddleLayerDecoderSplit.

Reference computation (per parent cluster p of N=4096, K=32 points):
    rel = x[:, :128] @ W_dec + b_dec            # [N, 96] -> [N*K, 3]
    h0  = concat(feat[cluster], rel_pts)        # [N*K, 259]
    h   = relu(relu(relu(h0@W1+b1)@W2+b2)@W3+b3)  # [N*K, 256]
Outputs: (rel_pts [N*K,3], h [N*K,256], cluster [N*K] int32)

Sharding: data-parallel over parent rows, 512 parents per core x 8 cores.
Weights replicated. cluster is input-independent -> computed host-side.

Per-core dataflow (all activations kept TRANSPOSED [feature, points] so no
per-layer transposes are needed; W1/W2/W3 natural [K,M] layout == lhsT):
  - x [512,384] loaded, PE-transposed to xT [384,512]
  - AT = W1a.T @ featT + b1  [512,512]  (the feat-dup factorization: the
    first-layer contribution of parent features is shared by all 32 points
    of a parent -> compute once per parent, broadcast-add per k)
  - rel_k [3,32,512] bf16: per k, one matmul with a 3-column slice of W_dec
    as the stationary puts rel_kT at partitions 0:2 (+ b_dec bias on evac)
  - per k (32 iters):
      psum = W1b.T @ rel_kT  (K=3 bf16 matmul)
      h1T  = relu(psum + AT)              (DVE add, ACT relu; b1 in AT)
      h2T  = relu(W2.T @ h1T + b2)        (bf16 weights/activations)
      h3   = relu(h2T.T-slices @ W3 + b3) (row-major; b3 injected via a
                                           K=1 ones-row matmul)
The h-chain matmuls run in bf16 (1 cycle/row vs ~2 for fp32r); the rel
output path stays fp32r for full precision. exec ~299us on 8 cores.
"""

import time

import numpy as np

import concourse.bass as bass
import concourse.tile as tile
from concourse import bacc, mybir
from concourse.bass import ts
from concourse.bass_utils import run_bass_kernel_spmd
from concourse.masks import make_identity

F32 = mybir.dt.float32
F32R = mybir.dt.float32r
BF16 = mybir.dt.bfloat16
AF = mybir.ActivationFunctionType
ALU = mybir.AluOpType

N_PARENTS = 4096
KPTS = 32          # points per parent
NEIGH = 128
FEAT = 256
NCORES = 8
NPL = N_PARENTS // NCORES          # 512 parents per core
PTS = NPL * KPTS                   # 16384 points per core
D1, D2, D3 = 512, 512, 256


def _bcast_ap(ap, n):
    """Prepend a 0-stride dim of size n (partition broadcast for DMA)."""
    return bass.AP(tensor=ap.tensor, offset=ap.offset, ap=[[0, n], *ap.ap])


def build_nc():
    nc = bacc.Bacc("TRN2", target_bir_lowering=False)

    x_d = nc.dram_tensor("x", [NPL, NEIGH + FEAT], F32, kind="ExternalInput")
    wdec_d = nc.dram_tensor("wdec", [NEIGH, KPTS * 3], F32, kind="ExternalInput")
    bdec_d = nc.dram_tensor("bdec", [KPTS * 3], F32, kind="ExternalInput")
    w1_d = nc.dram_tensor("w1", [FEAT + 3, D1], F32, kind="ExternalInput")
    b1_d = nc.dram_tensor("b1", [D1], F32, kind="ExternalInput")
    w2_d = nc.dram_tensor("w2", [D1, D2], F32, kind="ExternalInput")
    b2_d = nc.dram_tensor("b2", [D2], F32, kind="ExternalInput")
    w3_d = nc.dram_tensor("w3", [D2, D3], F32, kind="ExternalInput")
    b3_d = nc.dram_tensor("b3", [D3], F32, kind="ExternalInput")
    rel_d = nc.dram_tensor("rel", [PTS, 3], F32, kind="ExternalOutput")
    h_d = nc.dram_tensor("h", [PTS, D3], F32, kind="ExternalOutput")

    with tile.TileContext(nc) as tc:
        with (
            tc.tile_pool(name="consts", bufs=1) as consts,
            tc.tile_pool(name="work", bufs=4) as work,
            tc.tile_pool(name="ps", bufs=4, space="PSUM") as psp,
            tc.tile_pool(name="psB", bufs=2, space="PSUM") as pspB,
            tc.tile_pool(name="psC", bufs=2, space="PSUM") as pspC,
        ):
            # ---------------- constants / weights ----------------
            ident = consts.tile([128, 128], F32)
            make_identity(nc, ident[:, :])

            x_sb = consts.tile([128, 4, NEIGH + FEAT], F32)   # [p, pc, f]
            nc.sync.dma_start(
                out=x_sb[:, :, :],
                in_=x_d[:, :].rearrange("(pc p) f -> p pc f", p=128),
            )

            wdec_sb = consts.tile([128, KPTS * 3], F32R)
            nc.gpsimd.dma_start(out=wdec_sb[:, :], in_=wdec_d[:, :])

            # bdec staged [3, 32]: bdec32[j, k] = bdec[3k+j] (32 tiny DMAs --
            # partition-strided gather DMAs proved unreliable)
            bdec32 = consts.tile([3, KPTS], F32)
            for k in range(KPTS):
                nc.sync.dma_start(
                    out=bdec32[0:3, k : k + 1],
                    in_=bdec_d[3 * k : 3 * k + 3].rearrange("(j o) -> j o", o=1),
                )
            # bdec broadcast across partitions (free-dim bias for row-major rel)
            bdec_bc = consts.tile([128, KPTS * 3], F32)
            nc.sync.dma_start(out=bdec_bc[:, :], in_=_bcast_ap(bdec_d[:], 128))

            w1a_sb = consts.tile([128, 2, D1], F32R)           # [p, kc, m]
            nc.gpsimd.dma_start(
                out=w1a_sb[:, :, :],
                in_=w1_d[0:FEAT, :].rearrange("(kc p) m -> p kc m", p=128),
            )
            # last-3 rows of W1 in bf16 (the K=3 matmuls run in bf16)
            w1b0 = consts.tile([3, D1], BF16)
            nc.gpsimd.dma_start(out=w1b0[:, :], in_=w1_d[FEAT : FEAT + 3, :])
            w2_sb = consts.tile([128, 4, D2], BF16)
            nc.gpsimd.dma_start(
                out=w2_sb[:, :, :],
                in_=w2_d[:, :].rearrange("(kc p) m -> p kc m", p=128),
            )
            w3_sb = consts.tile([128, 4, D3], BF16)
            nc.gpsimd.dma_start(
                out=w3_sb[:, :, :],
                in_=w3_d[:, :].rearrange("(kc p) m -> p kc m", p=128),
            )
            b1_sb = consts.tile([128, 4], F32)
            nc.sync.dma_start(
                out=b1_sb[:, :], in_=b1_d[:].rearrange("(mc p) -> p mc", p=128)
            )
            b2_sb = consts.tile([128, 4], F32)
            nc.sync.dma_start(
                out=b2_sb[:, :], in_=b2_d[:].rearrange("(mc p) -> p mc", p=128)
            )

            # ---------------- transpose input: xT [384, 512] ----------------
            xT = consts.tile([128, 3, NPL], F32R)              # [q, fc, p]
            for pc in range(4):
                for fc in range(3):
                    pt = psp.tile([128, 128], F32, tag="ps")
                    nc.tensor.transpose(
                        pt[:, :], x_sb[:, pc, ts(fc, 128)], ident[:, :]
                    )
                    nc.vector.tensor_copy(out=xT[:, fc, ts(pc, 128)], in_=pt[:, :])

            # bf16 copies for the rel_k production matmuls
            wdec_bf = consts.tile([128, KPTS * 3], BF16)
            nc.vector.tensor_copy(out=wdec_bf[:, :], in_=wdec_sb[:, :])
            neigh_bf = consts.tile([128, NPL], BF16)
            nc.vector.tensor_copy(out=neigh_bf[:, :], in_=xT[:, 0, :])

            # ---------------- rel_k slabs [3, 32, 512] in bf16 ----------------
            # rel_k[j, k, p] = rel[p, 3k+j] + bdec[3k+j]; lhsT is a 3-column
            # slice of wdec -> output lands at partitions 0:2, M=3
            rel_k = consts.tile([3, KPTS, NPL], BF16)
            for k in range(KPTS):
                pt = pspC.tile([3, NPL], F32, tag="psC")
                nc.tensor.matmul(
                    pt[:, :],
                    wdec_bf[:, 3 * k : 3 * k + 3],
                    neigh_bf[:, :],
                )
                nc.vector.tensor_scalar_add(
                    out=rel_k[:, k, :], in0=pt[:, :], scalar1=bdec32[:, k : k + 1]
                )

            # ---------------- rel row-major output ----------------
            rel_rm = consts.tile([128, 4, KPTS * 3], F32)     # [p, pc, (k j)]
            rel_view = rel_d[:, :].rearrange("(p a) b -> p (a b)", a=KPTS)
            for pc in range(4):
                pt = psp.tile([128, KPTS * 3], F32, tag="ps")
                nc.tensor.matmul(
                    pt[:, :],
                    xT[:, 0, ts(pc, 128)],
                    wdec_sb[:, :],
                )
                nc.vector.tensor_tensor(
                    out=rel_rm[:, pc, :], in0=pt[:, :], in1=bdec_bc[:, :], op=ALU.add
                )
                nc.sync.dma_start(out=rel_view[ts(pc, 128), :], in_=rel_rm[:, pc, :])

            # ---------------- AT = W1a.T @ featT + b1  [512, 512] ----------------
            at_b = consts.tile([128, 4, NPL], F32)           # [m, mc, p]
            for mc in range(4):
                pt = psp.tile([128, NPL], F32, tag="ps")
                for kc in range(2):
                    nc.tensor.matmul(
                        pt[:, :],
                        w1a_sb[:, kc, ts(mc, 128)],
                        xT[:, 1 + kc, :],
                        start=(kc == 0),
                        stop=(kc == 1),
                    )
                nc.vector.tensor_scalar_add(
                    out=at_b[:, mc, :], in0=pt[:, :], scalar1=b1_sb[:, mc : mc + 1]
                )

            # ones/b3 rows in bf16 for the L3 bias-injection matmul
            ones_r = consts.tile([1, 128], BF16)
            ones_f = consts.tile([1, 128], F32)
            nc.vector.memset(ones_f[:, :], 1.0)
            nc.vector.tensor_copy(out=ones_r[:, :], in_=ones_f[:, :])
            b3_row = consts.tile([1, D3], BF16)
            nc.gpsimd.dma_start(
                out=b3_row[:, :], in_=b3_d[:].rearrange("(o d) -> o d", o=1)
            )

            # ---------------- main loop over k ----------------
            h_view = h_d[:, :].rearrange("(p k) d -> k p d", k=KPTS)
            for k in range(KPTS):
                # h1T_k = relu(AT + W1b.T @ rel_kT)  [512, 512]
                h1 = work.tile([128, 4, NPL], BF16, tag="h1")
                for mc in range(4):
                    pt = psp.tile([128, NPL], F32, tag="ps")
                    nc.tensor.matmul(
                        pt[:, :],
                        w1b0[:, ts(mc, 128)],
                        rel_k[:, k, :],
                    )
                    nc.vector.tensor_tensor(
                        out=pt[:, :], in0=pt[:, :], in1=at_b[:, mc, :], op=ALU.add
                    )
                    nc.scalar.activation(
                        out=h1[:, mc, :], in_=pt[:, :], func=AF.Relu
                    )

                # h2T_k = relu(W2.T @ h1T + b2)  [512, 512]
                h2 = work.tile([128, 4, NPL], BF16, tag="h2")
                for mc in range(4):
                    pt = pspB.tile([128, NPL], F32, tag="psB")
                    for kc in range(4):
                        nc.tensor.matmul(
                            pt[:, :],
                            w2_sb[:, kc, ts(mc, 128)],
                            h1[:, kc, :],
                            start=(kc == 0),
                            stop=(kc == 3),
                        )
                    nc.scalar.activation(
                        out=h2[:, mc, :],
                        in_=pt[:, :],
                        func=AF.Relu,
                        bias=b2_sb[:, mc : mc + 1],
                    )

                # h3_k = relu(h2T.T @ W3 + b3) row-major [512, 256]
                # b3 is injected via a K=1 ones-row matmul (bias varies along
                # the free dim here, which ACT bias cannot express)
                h3 = work.tile([128, 4, D3], F32, tag="h3")
                for rc in range(4):
                    pt = pspC.tile([128, D3], F32, tag="psC")
                    nc.tensor.matmul(
                        pt[:, :], ones_r[:, :], b3_row[:, :],
                        start=True, stop=False,
                    )
                    for kc in range(4):
                        nc.tensor.matmul(
                            pt[:, :],
                            h2[:, kc, ts(rc, 128)],
                            w3_sb[:, kc, :],
                            start=False,
                            stop=(kc == 3),
                        )
                    nc.vector.tensor_relu(out=h3[:, rc, :], in_=pt[:, :])
                    nc.sync.dma_start(
                        out=h_view[k, ts(rc, 128), :], in_=h3[:, rc, :]
                    )

    nc.compile()
    return nc


_NC_CACHE = None


def _get_nc():
    global _NC_CACHE
    if _NC_CACHE is None:
        _NC_CACHE = build_nc()
    return _NC_CACHE


def kernel(input_features, W_dec, b_dec, W1, b1, W2, b2, W3, b3):
    x = np.ascontiguousarray(np.asarray(input_features, dtype=np.float32))
    args = {
        "wdec": np.ascontiguousarray(np.asarray(W_dec, dtype=np.float32)),
        "bdec": np.ascontiguousarray(np.asarray(b_dec, dtype=np.float32)),
        "w1": np.ascontiguousarray(np.asarray(W1, dtype=np.float32)),
        "b1": np.ascontiguousarray(np.asarray(b1, dtype=np.float32)),
        "w2": np.ascontiguousarray(np.asarray(W2, dtype=np.float32)),
        "b2": np.ascontiguousarray(np.asarray(b2, dtype=np.float32)),
        "w3": np.ascontiguousarray(np.asarray(W3, dtype=np.float32)),
        "b3": np.ascontiguousarray(np.asarray(b3, dtype=np.float32)),
    }
    nc = _get_nc()
    in_maps = [
        {"x": np.ascontiguousarray(x[c * NPL : (c + 1) * NPL]), **args}
        for c in range(NCORES)
    ]
    # the first execute after a profiled session occasionally finds the
    # device wedged (NRT_EXEC_UNIT_UNRECOVERABLE); it recovers on retry
    for attempt in range(3):
        try:
            res = run_bass_kernel_spmd(nc, in_maps, list(range(NCORES)))
            break
        except Exception:
            if attempt == 2:
                raise
            time.sleep(5)
    rel = np.concatenate([res.results[c]["rel"] for c in range(NCORES)], axis=0)
    h = np.concatenate([res.results[c]["h"] for c in range(NCORES)], axis=0)
    cluster = np.repeat(np.arange(N_PARENTS, dtype=np.int32), KPTS)
    return rel, h, cluster
